# revision 41
# baseline (speedup 1.0000x reference)
"""Trainium2 Bass kernel for dense sigmoid-masked causal attention.

Problem (full shapes):
    x [B=2, N=2048, D=2048], W_qkv [D, 3D], b_qkv [3D], W_out [D, D],
    b_out [D], causal_mask [H=16, N, N]
    out = softmax((q k^T / sqrt(hd)) * sigmoid(mask)) v @ W_out + b_out

Sharding over 8 NeuronCores: 2-way data parallel on batch x 4-way tensor
parallel on heads (4 heads per core). Each core computes its partial
out-projection; the host sums the 4 partials per batch element.

Single fused pipeline (no phase barrier), "transposed scores" orientation:
    - sigmoid(mask) is precomputed on the HOST (it depends only on the mask
      input) and uploaded as sig^T quantized to uint8 (sigma in (0,1):
      absolute step 1/510; the 1/255 scale folds into the device exp's
      scale operand). Removes all device sigmoid work, ACT-table switches,
      and half the mask upload bytes (per-dispatch staging dominates the
      wall dispatch time under axon).
    - per head h: project qT/kT (out [cols, tokens]) and v (natural layout)
      from xT; head h+1's projection matmuls are interleaved into head h's
      four attention groups ("filler" queue) so the PE never idles between
      "phases". A prologue-only 6-buffer PSUM pool lets six projection
      chains pipeline against the initial DMA.
    - group (h, qc): 16 score matmuls [keys,512q] -> PSUM (4 cycling
      banks); mask-mul: quarters 0,1 via ACT copy PSUM->bf16 + Pool mul
      (GPSIMD cannot read PSUM), quarters 2,3 via DVE muls directly from
      PSUM; exp on ACT (scale=1/255) into attn [128, 512q, 16kc]
      (kc innermost); av matmuls read strided attn[:, :, kc] views.
    - softmax denominator: two bf16 pair-add levels (DVE 2x fast mode,
      nothing blocks >2.2us) + f32 X-reduce + gpsimd partition_all_reduce
      + DVE reciprocal; normalization (deferred one group; immediate in the
      head-3 pass) multiplies the av PSUM by the all-partition reciprocal.
      No PE matmuls are spent on den/broadcast.
    - out-projection (per 512-query stripe, all 4 heads) is interleaved
      into the head-3 pass one group behind its stripe's final norm;
      only stripe 3 drains in the tail. Output is bf16 partials (halves
      store traffic + staging); the host sums the four partials in f32.
    - PE does nothing but real matmuls: qkv proj + scores + av + out-proj
      (328us of pure matmul at bf16 peak; cost model ~358-398us/core).
"""

import functools

import numpy as np

B = 2
N = 2048
D = 2048
H = 16
HD = 128
HPC = 4  # heads per core
NCORES = 8
KC = D // 128  # 16 contraction chunks
Q = 512  # queries per group
ALPHA = 1.0 / float(np.sqrt(HD))
W3 = 3 * HD  # per-head wqkv column block (q|k|v)

# emission-time instruction labels (debug/profiling aid; harmless in prod)
LABELS = {}


def _lab(inst, label):
    try:
        LABELS[inst.ins.name] = label
    except Exception:
        pass
    return inst


@functools.lru_cache(maxsize=4)
def _build_program(zero_bias: bool, repeat: int = 1):
    import concourse.bass as bass  # noqa: F401
    import concourse.mybir as mybir
    import concourse.tile as tile
    from concourse import bacc

    f32 = mybir.dt.float32
    bf16 = mybir.dt.bfloat16

    # Bacc (not plain Bass): its compile() pass converts Tile's multi-sem
    # waits into event semaphores — walrus rejects raw multi-wait
    # instructions ("Too many sync wait commands").
    nc = bacc.Bacc("TRN2", target_bir_lowering=False, debug=False)

    xT_d = nc.declare_dram_parameter("xT", [D, N], bf16, isOutput=False)
    wq_d = nc.declare_dram_parameter("wqkvh", [HPC, D, W3], bf16, isOutput=False)
    bq_d = nc.declare_dram_parameter("bqkvh", [1, HPC * W3], bf16, isOutput=False)
    u8 = mybir.dt.uint8
    sig_d = nc.declare_dram_parameter("sigT", [HPC, N, N], u8, isOutput=False)
    wout_d = nc.declare_dram_parameter("wout", [HPC * HD, D], bf16, isOutput=False)
    bout_d = nc.declare_dram_parameter("bout", [1, D], bf16, isOutput=False)
    # bf16 output halves the per-dispatch staging + store traffic; the host
    # sums the four per-core partials in f32
    out_d = nc.declare_dram_parameter("out", [N, D], bf16, isOutput=True)

    with tile.TileContext(nc) as tc:
        for _rep in range(repeat):
            _emit_pipeline(
                nc, tc, mybir, zero_bias,
                xT_d, wq_d, bq_d, sig_d, wout_d, bout_d, out_d,
            )

    nc.compile()
    return nc


def _emit_pipeline(nc, tc, mybir, zero_bias, xT_d, wq_d, bq_d, sig_d, wout_d,
                   bout_d, out_d):
    from concourse import bass_isa

    f32 = mybir.dt.float32
    bf16 = mybir.dt.bfloat16
    Act = mybir.ActivationFunctionType
    Radd = bass_isa.ReduceOp.add
    X = mybir.AxisListType.X

    xT_r = xT_d.rearrange("(c p) n -> p c n", p=128)
    wq_r = [wq_d[h, :, :].rearrange("(c p) n -> p c n", p=128) for h in range(HPC)]
    sig_r = [sig_d[h, :, :].rearrange("(c p) q -> p c q", p=128) for h in range(HPC)]
    wout_r = wout_d.rearrange("(c p) n -> p c n", p=128)

    const = tc.alloc_tile_pool(name="const", bufs=1)
    persist = tc.alloc_tile_pool(name="persist", bufs=1)
    qkp = tc.alloc_tile_pool(name="qkp", bufs=2)
    vp = tc.alloc_tile_pool(name="vp", bufs=2)
    sigp = tc.alloc_tile_pool(name="sigp", bufs=6)
    attnp = tc.alloc_tile_pool(name="attnp", bufs=2)
    mskdp = tc.alloc_tile_pool(name="mskdp", bufs=2)
    denp = tc.alloc_tile_pool(name="denp", bufs=2)
    recipp = tc.alloc_tile_pool(name="recipp", bufs=2)
    ostp = tc.alloc_tile_pool(name="ostp", bufs=4)
    projps = tc.alloc_tile_pool(name="projps", bufs=2, space="PSUM")
    # prologue-only PSUM pool: 6 projection chains in flight while the
    # initial xT/wqkv DMA streams in; released before spsp/opsp exist
    prologps = tc.alloc_tile_pool(name="prologps", bufs=6, space="PSUM")
    # xT/wqkv pool allocated last so releasing it frees space for wout
    xtp = tc.alloc_tile_pool(name="xtp", bufs=1)
    wqp = tc.alloc_tile_pool(name="wqp", bufs=2)

    if not zero_bias:
        ones_bf = const.tile([128, Q], bf16)
        nc.vector.memset(ones_bf, 1.0)
        bq_sb = const.tile([1, HPC * W3], bf16)
        nc.sync.dma_start(out=bq_sb, in_=bq_d[:, :])

    oT = [persist.tile([128, N], bf16, name=f"oT{h}") for h in range(HPC)]

    # ---------------- DMA prologue: xT + wqkv[h0] interleaved -------------
    xT_sb = xtp.tile([128, KC, N], bf16)
    wq_t = {0: wqp.tile([128, KC, W3], bf16, name="wq", tag="wq")}
    for kc in range(KC):
        nc.sync.dma_start(out=xT_sb[:, kc, :], in_=xT_r[:, kc, :])
        nc.sync.dma_start(out=wq_t[0][:, kc, :], in_=wq_r[0][:, kc, :])

    qT, kT, vh = {}, {}, {}
    sig_tiles = {}

    def emit_sig_dma(gi):
        h, qc = divmod(gi, 4)
        qs = slice(qc * Q, (qc + 1) * Q)
        quads = []
        for qt in range(4):
            sg = sigp.tile([128, 4, Q], mybir.dt.uint8, name="sig", tag="sig")
            nc.sync.dma_start(out=sg, in_=sig_r[h][:, 4 * qt : 4 * qt + 4, qs])
            quads.append(sg)
        sig_tiles[gi] = quads

    def emit_wq_dma(h):
        wq_t[h] = wqp.tile([128, KC, W3], bf16, name="wq", tag="wq")
        nc.sync.dma_start(out=wq_t[h], in_=wq_r[h][:, :, :])

    # ---- projection chain emitters (the PE "filler" work queue) ----------
    def chain_qk(h, t, col0, pool=None):
        # one [128, 512] output chunk of qT/kT head h (col0 0:q, HD:k)
        dst = qT if col0 == 0 else kT
        pool = pool or projps
        ps = pool.tile([128, Q], f32, name="projps", tag="projps")
        for kc in range(KC):
            _lab(nc.tensor.matmul(
                ps,
                lhsT=wq_t[h][:, kc, col0 : col0 + HD],
                rhs=xT_sb[:, kc, t * Q : (t + 1) * Q],
                start=(kc == 0),
                stop=(kc == KC - 1) and zero_bias,
            ), "proj_qk")
        if not zero_bias:
            c0 = h * W3 + col0
            nc.tensor.matmul(
                ps, lhsT=bq_sb[0:1, c0 : c0 + HD],
                rhs=ones_bf[0:1, :], start=False, stop=True,
            )
        nc.vector.tensor_copy(dst[h][:, t * Q : (t + 1) * Q], ps)

    def chain_v(h, tok, pool=None):
        # one [128-token, 128-col] chunk of v head h (natural layout)
        pool = pool or projps
        ps = pool.tile([128, Q], f32, name="projps", tag="projps")
        for kc in range(KC):
            _lab(nc.tensor.matmul(
                ps[:, 0:HD],
                lhsT=xT_sb[:, kc, tok * HD : (tok + 1) * HD],
                rhs=wq_t[h][:, kc, 2 * HD : 3 * HD],
                start=(kc == 0),
                stop=(kc == KC - 1) and zero_bias,
            ), "proj_v")
        if not zero_bias:
            c0 = h * W3 + 2 * HD
            nc.tensor.matmul(
                ps[:, 0:HD], lhsT=ones_bf[0:1, 0:HD],
                rhs=bq_sb[0:1, c0 : c0 + HD], start=False, stop=True,
            )
        nc.scalar.copy(vh[h][:, tok, :], ps[:, 0:HD])

    def chain_outproj(qc, t2, cc):
        # one [128-query, 512-outcol] chunk of the out-projection
        t0 = qc * Q + t2 * 128
        cs = slice(cc * Q, (cc + 1) * Q)
        ps = projps.tile([128, Q], f32, name="projps", tag="projps")
        for hh in range(HPC):
            _lab(nc.tensor.matmul(
                ps, lhsT=oT[hh][:, t0 : t0 + 128], rhs=wout_sb[:, hh, cs],
                start=(hh == 0), stop=(hh == HPC - 1) and zero_bias,
            ), "outproj")
        if not zero_bias:
            nc.tensor.matmul(
                ps, lhsT=ones_bf[0:1, 0:128], rhs=bout_sb[0:1, cs],
                start=False, stop=True,
            )
        ost = ostp.tile([128, Q], bf16, name="ost", tag="ost")
        if (t2 * 4 + cc) % 2 == 0:
            nc.scalar.copy(ost, ps)
        else:
            nc.vector.tensor_copy(ost, ps)
        nc.sync.dma_start(out=out_d[t0 : t0 + 128, cs], in_=ost)

    # FIFO of pending filler chains: (cost_us, emit_fn)
    filler = []

    held_back = []

    def push_proj(h, early_pool=None):
        qT[h] = qkp.tile([128, N], bf16, name="qT", tag="qT")
        kT[h] = qkp.tile([128, N], bf16, name="kT", tag="kT")
        vh[h] = vp.tile([128, KC, HD], bf16, name="vh", tag="vh")
        ep = early_pool
        for t in range(4):
            filler.append((3.4, lambda t=t: chain_qk(h, t, HD, ep)))  # kT 1st
        filler.append((3.4, lambda: chain_qk(h, 0, 0, ep)))  # qT chunk 0
        for tok in range(KC):
            filler.append((0.9, lambda tok=tok: chain_v(h, tok, ep)))
        for t in range(1, 4):
            if h == 3 and t >= 2:
                # head-3 late q chunks: fill the (otherwise proj-less)
                # head-3 pass; chunk t is needed only by group (3, t)
                held_back.append((3.4, lambda t=t: chain_qk(h, t, 0)))
            else:
                filler.append((3.4, lambda t=t: chain_qk(h, t, 0)))

    def push_outproj(qc):
        for t2 in range(4):
            for cc in range(4):
                filler.append((0.9, lambda t2=t2, cc=cc: chain_outproj(qc, t2, cc)))

    def pop_filler(budget_us):
        used = 0.0
        while filler and used < budget_us:
            cost, fn = filler.pop(0)
            fn()
            used += cost

    # ---------------- prologue: project head 0 ----------------------------
    push_proj(0, early_pool=prologps)
    # emit k chunks + q chunk 0 + all v now; the rest fills group slots
    pop_filler(3.4 * 5 + 0.9 * 16 - 0.1)
    prologps.release()
    spsp = tc.alloc_tile_pool(name="spsp", bufs=4, space="PSUM")
    opsp = tc.alloc_tile_pool(name="opsp", bufs=2, space="PSUM")
    emit_sig_dma(0)
    emit_wq_dma(1)

    # ---------------- 16 attention groups ---------------------------------
    deferred = None  # (ops_tile, h, qs, attn_tile)

    def deferred_den(d):
        # den = sum over keys of attn: two bf16 pair-add levels (DVE 2x fast
        # mode, max 2.2us per op so muls are never blocked long), f32
        # X-reduce over 4 chunks, Pool allreduce
        d_ops, d_h, d_qs, d_attn = d
        t8 = denp.tile([128, Q, 8], bf16, name="t8", tag="t8", bufs=1)
        nc.vector.tensor_add(t8, d_attn[:, :, 0:8], d_attn[:, :, 8:16])
        t4 = denp.tile([128, Q, 4], bf16, name="t4", tag="t4", bufs=1)
        nc.vector.tensor_add(t4, t8[:, :, 0:4], t8[:, :, 4:8])
        den_sb = denp.tile([128, Q], f32, name="den", tag="den", bufs=1)
        nc.vector.tensor_reduce(den_sb, t4, axis=X, op=mybir.AluOpType.add)
        denr_sb = denp.tile([128, Q], f32, name="denr", tag="denr", bufs=1)
        nc.gpsimd.partition_all_reduce(denr_sb, den_sb, channels=128, reduce_op=Radd)
        return denr_sb

    def deferred_norm(d, denr_sb):
        # DVE recip + normalization (after this group's DVE muls)
        d_ops, d_h, d_qs, d_attn = d
        recip_sb = recipp.tile([128, Q], f32, name="recip", tag="recip")
        nc.vector.reciprocal(recip_sb, denr_sb)
        nc.vector.tensor_mul(oT[d_h][:, d_qs], d_ops, recip_sb)

    def finish_deferred(d):
        deferred_norm(d, deferred_den(d))

    for gi in range(16):
        h, qc = divmod(gi, 4)
        qs = slice(qc * Q, (qc + 1) * Q)

        if gi + 1 < 16:
            emit_sig_dma(gi + 1)
        if qc == 0 and 0 < h < 3:
            emit_wq_dma(h + 1)
        if qc == 0 and h < 3:
            push_proj(h + 1)
        if h == 3 and qc == 0:
            # drain all xT/wqkv readers (incl. held-back chains), free
            # them, and bring in wout for the out-projection
            filler.extend(held_back)
            pop_filler(100.0)
            wqp.release()
            xtp.release()
            woutp = tc.alloc_tile_pool(name="woutp", bufs=1)
            wout_sb = woutp.tile([128, HPC, D], bf16)
            nc.sync.dma_start(out=wout_sb, in_=wout_r)
            if not zero_bias:
                bout_sb = woutp.tile([1, D], bf16)
                nc.sync.dma_start(out=bout_sb, in_=bout_d[:, :])
        if h == 3 and qc >= 1:
            # h3 norms are immediate (not deferred), so stripe qc-1 is
            # ready one group earlier; only stripe 3 remains for the tail
            push_outproj(qc - 1)

        # -- PE: scores first half | filler | second half | filler ---------
        sps_t = []
        for kc in range(8):
            sps = spsp.tile([128, Q], f32, name="sps", tag="sps")
            _lab(nc.tensor.matmul(
                sps, lhsT=kT[h][:, kc * 128 : (kc + 1) * 128],
                rhs=qT[h][:, qs], start=True, stop=True,
            ), "score_a")
            sps_t.append(sps)
        pop_filler(5.1)
        for kc in range(8, KC):
            sps = spsp.tile([128, Q], f32, name="sps", tag="sps")
            _lab(nc.tensor.matmul(
                sps, lhsT=kT[h][:, kc * 128 : (kc + 1) * 128],
                rhs=qT[h][:, qs], start=True, stop=True,
            ), "score_b")
            sps_t.append(sps)
        pop_filler(5.1)

        # -- mask-muls (GPSIMD cannot touch PSUM on real HW):
        # quarters 0,1: ACT copies PSUM->bf16 SBUF, Pool muls in-place
        # quarters 2,3: DVE muls directly from PSUM
        mskd_q = []
        for qt in range(2):
            mq = mskdp.tile([128, 4, Q], bf16, name="mskd", tag="mskd")
            for k2 in range(4):
                nc.scalar.copy(mq[:, k2, :], sps_t[4 * qt + k2])
            mskd_q.append(mq)
        for qt in range(2):
            for k2 in range(4):
                nc.gpsimd.tensor_mul(
                    mskd_q[qt][:, k2, :],
                    mskd_q[qt][:, k2, :],
                    sig_tiles[gi][qt][:, k2, :],
                )
        for qt in range(2, 4):
            mq = mskdp.tile([128, 4, Q], bf16, name="mskd", tag="mskd")
            for k2 in range(4):
                kc = 4 * qt + k2
                nc.vector.tensor_mul(
                    mq[:, k2, :], sps_t[kc], sig_tiles[gi][qt][:, k2, :]
                )
            mskd_q.append(mq)
        del sig_tiles[gi]
        # -- deferred den + normalization for g-1 (after DVE muls) ---------
        if deferred is not None:
            deferred_norm(deferred, deferred_den(deferred))

        # -- ACT: exp eighths into strided attn (finer av unblocking) ------
        attn_t = attnp.tile([128, Q, KC], bf16, name="attn", tag="attn")
        for qt in range(4):
            nc.scalar.activation(
                attn_t[:, :, 4 * qt : 4 * qt + 4],
                mskd_q[qt].rearrange("p c q -> p q c"),
                Act.Exp,
                scale=1.0 / 255.0,
            )

        # -- PE: av (strided rhs) ------------------------------------------
        ops_t = opsp.tile([128, Q], f32, name="ops", tag="ops")
        for kc in range(KC):
            _lab(nc.tensor.matmul(
                ops_t, lhsT=vh[h][:, kc, :], rhs=attn_t[:, :, kc],
                start=(kc == 0), stop=(kc == KC - 1),
            ), "av")
        if h == 3:
            # head-3 pass: finish this group's den+norm immediately so its
            # out-projection stripe unblocks one group later
            finish_deferred((ops_t, h, qs, attn_t))
            deferred = None
        else:
            deferred = (ops_t, h, qs, attn_t)

    # ---------------- tail -------------------------------------------------
    assert deferred is None
    push_outproj(3)
    pop_filler(100.0)

    # release in reverse-allocation order (xtp/wqp already released)
    for p in (woutp, opsp, spsp, projps, ostp, recipp, denp, mskdp, attnp,
              sigp, vp, qkp, persist, const):
        p.release()


def _prep_in_maps(x, W_qkv, b_qkv, W_out, b_out, causal_mask):
    from concurrent.futures import ThreadPoolExecutor

    import ml_dtypes

    bf = ml_dtypes.bfloat16

    def _xT(b):
        return np.ascontiguousarray(x[b].T).astype(bf)

    def _sigT(h):
        # host-side sigmoid (input-only), f32 math, transposed, quantized to
        # u8 (sigma in (0,1): absolute step 1/255; the 1/255 scale is folded
        # into the device exp's scale operand)
        m = causal_mask[h]
        s = 1.0 / (1.0 + np.exp(-m, dtype=np.float32))
        return np.ascontiguousarray(np.rint(s.T * 255.0)).astype(np.uint8)

    def _wq(g):
        # head-major wqkv block [HPC, D, 384] for head group g
        h0 = g * HPC
        out = np.empty((HPC, D, W3), dtype=bf)
        for j in range(HPC):
            h = h0 + j
            cs = slice(h * HD, (h + 1) * HD)
            out[j, :, 0:HD] = (W_qkv[:, cs] * ALPHA).astype(bf)
            out[j, :, HD : 2 * HD] = W_qkv[:, D + h * HD : D + (h + 1) * HD].astype(bf)
            out[j, :, 2 * HD : 3 * HD] = W_qkv[
                :, 2 * D + h * HD : 2 * D + (h + 1) * HD
            ].astype(bf)
        return out

    with ThreadPoolExecutor(16) as ex:
        xT_f = [ex.submit(_xT, b) for b in range(B)]
        sig_f = [ex.submit(_sigT, h) for h in range(H)]
        wq_f = [ex.submit(_wq, g) for g in range(4)]
        xT = [f.result() for f in xT_f]
        sigT = [f.result() for f in sig_f]
        wqh = [f.result() for f in wq_f]

    sig_stack = [
        np.stack([sigT[g * HPC + j] for j in range(HPC)]) for g in range(4)
    ]
    in_maps = []
    for c in range(NCORES):
        b = c // 4
        g = c % 4
        h0 = g * HPC
        bq = np.empty((1, HPC, W3), dtype=bf)
        for j in range(HPC):
            h = h0 + j
            bq[0, j, 0:HD] = (b_qkv[h * HD : (h + 1) * HD] * ALPHA).astype(bf)
            bq[0, j, HD : 2 * HD] = b_qkv[D + h * HD : D + (h + 1) * HD].astype(bf)
            bq[0, j, 2 * HD :] = b_qkv[2 * D + h * HD : 2 * D + (h + 1) * HD].astype(bf)
        in_maps.append(
            {
                "xT": xT[b],
                "wqkvh": wqh[g],
                "bqkvh": bq.reshape(1, HPC * W3),
                "sigT": sig_stack[g],
                "wout": W_out[h0 * HD : (h0 + HPC) * HD, :].astype(bf),
                "bout": (b_out * 0.25).reshape(1, -1).astype(bf),
            }
        )
    return in_maps


def _zero_bias(b_qkv, b_out):
    return bool(not b_qkv.any() and not b_out.any())


def kernel(**inputs):
    x = np.asarray(inputs["x"], dtype=np.float32)
    W_qkv = np.asarray(inputs["W_qkv"], dtype=np.float32)
    b_qkv = np.asarray(inputs["b_qkv"], dtype=np.float32)
    W_out = np.asarray(inputs["W_out"], dtype=np.float32)
    b_out = np.asarray(inputs["b_out"], dtype=np.float32)
    causal_mask = np.asarray(inputs["causal_mask"], dtype=np.float32)

    from concourse.bass_utils import run_bass_kernel_spmd

    nc = _build_program(_zero_bias(b_qkv, b_out))
    in_maps = _prep_in_maps(x, W_qkv, b_qkv, W_out, b_out, causal_mask)
    res = run_bass_kernel_spmd(nc, in_maps, core_ids=list(range(NCORES)))

    out = np.zeros((B, N, D), dtype=np.float32)
    for c in range(NCORES):
        out[c // 4] += np.asarray(res.results[c]["out"]).astype(np.float32)
    return out


# revision 46
# speedup vs baseline: 1.0185x; 1.0185x over previous
"""Trainium2 Bass kernel for dense sigmoid-masked causal attention.

Problem (full shapes):
    x [B=2, N=2048, D=2048], W_qkv [D, 3D], b_qkv [3D], W_out [D, D],
    b_out [D], causal_mask [H=16, N, N]
    out = softmax((q k^T / sqrt(hd)) * sigmoid(mask)) v @ W_out + b_out

Sharding over 8 NeuronCores: 2-way data parallel on batch x 4-way tensor
parallel on heads (4 heads per core). Each core computes its partial
out-projection; the host sums the 4 partials per batch element.

Single fused pipeline (no phase barrier), "transposed scores" orientation:
    - sigmoid(mask) is precomputed on the HOST (it depends only on the mask
      input) and uploaded as sig^T quantized to uint8 (sigma in (0,1):
      absolute step 1/510; the 1/255 scale folds into the device exp's
      scale operand). Removes all device sigmoid work, ACT-table switches,
      and half the mask upload bytes (per-dispatch staging dominates the
      wall dispatch time under axon).
    - per head h: project qT/kT (out [cols, tokens]) and v (natural layout)
      from xT; head h+1's projection matmuls are interleaved into head h's
      four attention groups ("filler" queue) so the PE never idles between
      "phases". A prologue-only 6-buffer PSUM pool lets six projection
      chains pipeline against the initial DMA.
    - group (h, qc): 16 score matmuls [keys,512q] -> PSUM (4 cycling
      banks); mask-mul: quarters 0,1 via ACT copy PSUM->bf16 + Pool mul
      (GPSIMD cannot read PSUM), quarters 2,3 via DVE muls directly from
      PSUM; exp on ACT (scale=1/255) into attn [128, 512q, 16kc]
      (kc innermost); av matmuls read strided attn[:, :, kc] views.
    - softmax denominator: two bf16 pair-add levels (DVE 2x fast mode,
      nothing blocks >2.2us) + f32 X-reduce + gpsimd partition_all_reduce
      + DVE reciprocal; normalization (deferred one group; immediate in the
      head-3 pass) multiplies the av PSUM by the all-partition reciprocal.
      No PE matmuls are spent on den/broadcast.
    - out-projection (per 512-query stripe, all 4 heads) is interleaved
      into the head-3 pass one group behind its stripe's final norm;
      only stripe 3 drains in the tail. Output is bf16 partials (halves
      store traffic + staging); the host sums the four partials in f32.
    - PE does nothing but real matmuls: qkv proj + scores + av + out-proj
      (328us of pure matmul at bf16 peak; cost model ~358-398us/core).
"""

import functools

import numpy as np

B = 2
N = 2048
D = 2048
H = 16
HD = 128
HPC = 4  # heads per core
NCORES = 8
KC = D // 128  # 16 contraction chunks
Q = 512  # queries per group
ALPHA = 1.0 / float(np.sqrt(HD))
W3 = 3 * HD  # per-head wqkv column block (q|k|v)

# emission-time instruction labels (debug/profiling aid; harmless in prod)
LABELS = {}


def _lab(inst, label):
    try:
        LABELS[inst.ins.name] = label
    except Exception:
        pass
    return inst


@functools.lru_cache(maxsize=4)
def _build_program(zero_bias: bool, repeat: int = 1):
    import concourse.bass as bass  # noqa: F401
    import concourse.mybir as mybir
    import concourse.tile as tile
    from concourse import bacc

    f32 = mybir.dt.float32
    bf16 = mybir.dt.bfloat16

    # Bacc (not plain Bass): its compile() pass converts Tile's multi-sem
    # waits into event semaphores — walrus rejects raw multi-wait
    # instructions ("Too many sync wait commands").
    nc = bacc.Bacc("TRN2", target_bir_lowering=False, debug=False)

    xT_d = nc.declare_dram_parameter("xT", [D, N], bf16, isOutput=False)
    wq_d = nc.declare_dram_parameter("wqkvh", [HPC, D, W3], bf16, isOutput=False)
    bq_d = nc.declare_dram_parameter("bqkvh", [1, HPC * W3], bf16, isOutput=False)
    u8 = mybir.dt.uint8
    sig_d = nc.declare_dram_parameter("sigT", [HPC, N, N], u8, isOutput=False)
    wout_d = nc.declare_dram_parameter("wout", [HPC * HD, D], bf16, isOutput=False)
    bout_d = nc.declare_dram_parameter("bout", [1, D], bf16, isOutput=False)
    # bf16 output halves the per-dispatch staging + store traffic; the host
    # sums the four per-core partials in f32
    out_d = nc.declare_dram_parameter("out", [N, D], bf16, isOutput=True)

    with tile.TileContext(nc) as tc:
        for _rep in range(repeat):
            _emit_pipeline(
                nc, tc, mybir, zero_bias,
                xT_d, wq_d, bq_d, sig_d, wout_d, bout_d, out_d,
            )

    nc.compile()
    return nc


def _emit_pipeline(nc, tc, mybir, zero_bias, xT_d, wq_d, bq_d, sig_d, wout_d,
                   bout_d, out_d):
    from concourse import bass_isa

    f32 = mybir.dt.float32
    bf16 = mybir.dt.bfloat16
    Act = mybir.ActivationFunctionType
    Radd = bass_isa.ReduceOp.add
    X = mybir.AxisListType.X

    xT_r = xT_d.rearrange("(c p) n -> p c n", p=128)
    wq_r = [wq_d[h, :, :].rearrange("(c p) n -> p c n", p=128) for h in range(HPC)]
    sig_r = [sig_d[h, :, :].rearrange("(c p) q -> p c q", p=128) for h in range(HPC)]
    wout_r = wout_d.rearrange("(c p) n -> p c n", p=128)

    const = tc.alloc_tile_pool(name="const", bufs=1)
    persist = tc.alloc_tile_pool(name="persist", bufs=1)
    qkp = tc.alloc_tile_pool(name="qkp", bufs=2)
    vp = tc.alloc_tile_pool(name="vp", bufs=2)
    sigp = tc.alloc_tile_pool(name="sigp", bufs=6)
    attnp = tc.alloc_tile_pool(name="attnp", bufs=2)
    mskdp = tc.alloc_tile_pool(name="mskdp", bufs=2)
    denp = tc.alloc_tile_pool(name="denp", bufs=2)
    recipp = tc.alloc_tile_pool(name="recipp", bufs=2)
    ostp = tc.alloc_tile_pool(name="ostp", bufs=4)
    projps = tc.alloc_tile_pool(name="projps", bufs=2, space="PSUM")
    # prologue-only PSUM pool: 6 projection chains in flight while the
    # initial xT/wqkv DMA streams in; released before spsp/opsp exist
    prologps = tc.alloc_tile_pool(name="prologps", bufs=6, space="PSUM")
    # xT/wqkv pool allocated last so releasing it frees space for wout
    xtp = tc.alloc_tile_pool(name="xtp", bufs=1)
    wqp = tc.alloc_tile_pool(name="wqp", bufs=2)

    if not zero_bias:
        ones_bf = const.tile([128, Q], bf16)
        nc.vector.memset(ones_bf, 1.0)
        bq_sb = const.tile([1, HPC * W3], bf16)
        nc.sync.dma_start(out=bq_sb, in_=bq_d[:, :])

    oT = [persist.tile([128, N], bf16, name=f"oT{h}") for h in range(HPC)]

    # ---------------- DMA prologue: xT + wqkv[h0] interleaved -------------
    xT_sb = xtp.tile([128, KC, N], bf16)
    wq_t = {0: wqp.tile([128, KC, W3], bf16, name="wq", tag="wq")}
    for kc in range(KC):
        nc.sync.dma_start(out=xT_sb[:, kc, :], in_=xT_r[:, kc, :])
        nc.sync.dma_start(out=wq_t[0][:, kc, :], in_=wq_r[0][:, kc, :])

    qT, kT, vh = {}, {}, {}
    sig_tiles = {}

    def emit_sig_dma(gi):
        h, qc = divmod(gi, 4)
        qs = slice(qc * Q, (qc + 1) * Q)
        quads = []
        for qt in range(4):
            sg = sigp.tile([128, 4, Q], mybir.dt.uint8, name="sig", tag="sig")
            nc.sync.dma_start(out=sg, in_=sig_r[h][:, 4 * qt : 4 * qt + 4, qs])
            quads.append(sg)
        sig_tiles[gi] = quads

    def emit_wq_dma(h):
        wq_t[h] = wqp.tile([128, KC, W3], bf16, name="wq", tag="wq")
        nc.sync.dma_start(out=wq_t[h], in_=wq_r[h][:, :, :])

    # ---- projection chain emitters (the PE "filler" work queue) ----------
    def chain_qk(h, t, col0, pool=None):
        # one [128, 512] output chunk of qT/kT head h (col0 0:q, HD:k)
        dst = qT if col0 == 0 else kT
        pool = pool or projps
        ps = pool.tile([128, Q], f32, name="projps", tag="projps")
        for kc in range(KC):
            _lab(nc.tensor.matmul(
                ps,
                lhsT=wq_t[h][:, kc, col0 : col0 + HD],
                rhs=xT_sb[:, kc, t * Q : (t + 1) * Q],
                start=(kc == 0),
                stop=(kc == KC - 1) and zero_bias,
            ), "proj_qk")
        if not zero_bias:
            c0 = h * W3 + col0
            nc.tensor.matmul(
                ps, lhsT=bq_sb[0:1, c0 : c0 + HD],
                rhs=ones_bf[0:1, :], start=False, stop=True,
            )
        nc.vector.tensor_copy(dst[h][:, t * Q : (t + 1) * Q], ps)

    def chain_v(h, tok, pool=None):
        # one [128-token, 128-col] chunk of v head h (natural layout)
        pool = pool or projps
        ps = pool.tile([128, Q], f32, name="projps", tag="projps")
        for kc in range(KC):
            _lab(nc.tensor.matmul(
                ps[:, 0:HD],
                lhsT=xT_sb[:, kc, tok * HD : (tok + 1) * HD],
                rhs=wq_t[h][:, kc, 2 * HD : 3 * HD],
                start=(kc == 0),
                stop=(kc == KC - 1) and zero_bias,
            ), "proj_v")
        if not zero_bias:
            c0 = h * W3 + 2 * HD
            nc.tensor.matmul(
                ps[:, 0:HD], lhsT=ones_bf[0:1, 0:HD],
                rhs=bq_sb[0:1, c0 : c0 + HD], start=False, stop=True,
            )
        nc.scalar.copy(vh[h][:, tok, :], ps[:, 0:HD])

    def chain_outproj(qc, t2, cc):
        # one [128-query, 512-outcol] chunk of the out-projection
        t0 = qc * Q + t2 * 128
        cs = slice(cc * Q, (cc + 1) * Q)
        ps = projps.tile([128, Q], f32, name="projps", tag="projps")
        for hh in range(HPC):
            _lab(nc.tensor.matmul(
                ps, lhsT=oT[hh][:, t0 : t0 + 128], rhs=wout_sb[:, hh, cs],
                start=(hh == 0), stop=(hh == HPC - 1) and zero_bias,
            ), "outproj")
        if not zero_bias:
            nc.tensor.matmul(
                ps, lhsT=ones_bf[0:1, 0:128], rhs=bout_sb[0:1, cs],
                start=False, stop=True,
            )
        ost = ostp.tile([128, Q], bf16, name="ost", tag="ost")
        if (t2 * 4 + cc) % 2 == 0:
            nc.scalar.copy(ost, ps)
        else:
            nc.vector.tensor_copy(ost, ps)
        nc.sync.dma_start(out=out_d[t0 : t0 + 128, cs], in_=ost)

    # FIFO of pending filler chains: (cost_us, emit_fn)
    filler = []

    held_back = []

    def push_proj(h, early_pool=None):
        qT[h] = qkp.tile([128, N], bf16, name="qT", tag="qT")
        kT[h] = qkp.tile([128, N], bf16, name="kT", tag="kT")
        vh[h] = vp.tile([128, KC, HD], bf16, name="vh", tag="vh")
        ep = early_pool
        for t in range(4):
            filler.append((3.4, lambda t=t: chain_qk(h, t, HD, ep)))  # kT 1st
        filler.append((3.4, lambda: chain_qk(h, 0, 0, ep)))  # qT chunk 0
        for tok in range(KC):
            if h == 3 and tok >= 8:
                # fill the proj-less head-3 pass; av(3,0) consumes v
                # chunk-by-chunk so these land just in time
                held_back.append((0.9, lambda tok=tok: chain_v(h, tok, ep)))
            else:
                filler.append((0.9, lambda tok=tok: chain_v(h, tok, ep)))
        for t in range(1, 4):
            if h == 3 and t >= 2:
                # head-3 late q chunks: fill the (otherwise proj-less)
                # head-3 pass; chunk t is needed only by group (3, t)
                held_back.append((3.4, lambda t=t: chain_qk(h, t, 0)))
            else:
                filler.append((3.4, lambda t=t: chain_qk(h, t, 0)))

    def push_outproj(qc):
        for t2 in range(4):
            for cc in range(4):
                filler.append((0.9, lambda t2=t2, cc=cc: chain_outproj(qc, t2, cc)))

    def pop_filler(budget_us):
        used = 0.0
        while filler and used < budget_us:
            cost, fn = filler.pop(0)
            fn()
            used += cost

    # ---------------- prologue: project head 0 ----------------------------
    push_proj(0, early_pool=prologps)
    # emit k chunks + q chunk 0 + all v now; the rest fills group slots
    pop_filler(3.4 * 5 + 0.9 * 16 - 0.1)
    prologps.release()
    spsp = tc.alloc_tile_pool(name="spsp", bufs=4, space="PSUM")
    opsp = tc.alloc_tile_pool(name="opsp", bufs=2, space="PSUM")
    emit_sig_dma(0)
    emit_wq_dma(1)

    # ---------------- 16 attention groups ---------------------------------
    deferred = None  # (ops_tile, h, qs, attn_tile)

    def deferred_den(d):
        # den = sum over keys of attn: two bf16 pair-add levels (DVE 2x fast
        # mode, max 2.2us per op so muls are never blocked long), f32
        # X-reduce over 4 chunks, Pool allreduce
        d_ops, d_h, d_qs, d_attn = d
        t8 = denp.tile([128, Q, 8], bf16, name="t8", tag="t8", bufs=1)
        nc.vector.tensor_add(t8, d_attn[:, :, 0:8], d_attn[:, :, 8:16])
        t4 = denp.tile([128, Q, 4], bf16, name="t4", tag="t4", bufs=1)
        nc.vector.tensor_add(t4, t8[:, :, 0:4], t8[:, :, 4:8])
        den_sb = denp.tile([128, Q], f32, name="den", tag="den", bufs=1)
        nc.vector.tensor_reduce(den_sb, t4, axis=X, op=mybir.AluOpType.add)
        denr_sb = denp.tile([128, Q], f32, name="denr", tag="denr", bufs=1)
        nc.gpsimd.partition_all_reduce(denr_sb, den_sb, channels=128, reduce_op=Radd)
        return denr_sb

    def deferred_norm(d, denr_sb):
        # DVE recip + normalization (after this group's DVE muls)
        d_ops, d_h, d_qs, d_attn = d
        recip_sb = recipp.tile([128, Q], f32, name="recip", tag="recip")
        nc.vector.reciprocal(recip_sb, denr_sb)
        nc.vector.tensor_mul(oT[d_h][:, d_qs], d_ops, recip_sb)

    def finish_deferred(d):
        deferred_norm(d, deferred_den(d))

    for gi in range(16):
        h, qc = divmod(gi, 4)
        qs = slice(qc * Q, (qc + 1) * Q)

        if gi + 1 < 16:
            emit_sig_dma(gi + 1)
        if qc == 3 and h < 2:
            # fetch head h+2's weights one group before push_proj(h+2) pops
            # its first chains, hiding the 1.5MB transfer
            emit_wq_dma(h + 2)
        if qc == 0 and h < 3:
            push_proj(h + 1)
        if h == 3 and qc == 0:
            # drain all xT/wqkv readers (incl. held-back chains), free
            # them, and bring in wout for the out-projection
            filler.extend(held_back)
            pop_filler(100.0)
            wqp.release()
            xtp.release()
            woutp = tc.alloc_tile_pool(name="woutp", bufs=1)
            wout_sb = woutp.tile([128, HPC, D], bf16)
            nc.sync.dma_start(out=wout_sb, in_=wout_r)
            if not zero_bias:
                bout_sb = woutp.tile([1, D], bf16)
                nc.sync.dma_start(out=bout_sb, in_=bout_d[:, :])
        if h == 3 and qc >= 1:
            # h3 norms are immediate (not deferred), so stripe qc-1 is
            # ready one group earlier; only stripe 3 remains for the tail
            push_outproj(qc - 1)

        # -- PE: scores first half | filler | second half | filler ---------
        sps_t = []
        for kc in range(8):
            sps = spsp.tile([128, Q], f32, name="sps", tag="sps")
            _lab(nc.tensor.matmul(
                sps, lhsT=kT[h][:, kc * 128 : (kc + 1) * 128],
                rhs=qT[h][:, qs], start=True, stop=True,
            ), "score_a")
            sps_t.append(sps)
        pop_filler(5.1)
        for kc in range(8, KC):
            sps = spsp.tile([128, Q], f32, name="sps", tag="sps")
            _lab(nc.tensor.matmul(
                sps, lhsT=kT[h][:, kc * 128 : (kc + 1) * 128],
                rhs=qT[h][:, qs], start=True, stop=True,
            ), "score_b")
            sps_t.append(sps)
        ops_t = opsp.tile([128, Q], f32, name="ops", tag="ops")
        pop_filler(5.1)

        # -- mask-muls (GPSIMD cannot touch PSUM on real HW):
        # quarters 0,1: ACT copies PSUM->bf16 SBUF, Pool muls in-place
        # quarters 2,3: DVE muls directly from PSUM
        mskd_q = []
        for qt in range(2):
            mq = mskdp.tile([128, 4, Q], bf16, name="mskd", tag="mskd")
            for k2 in range(4):
                nc.scalar.copy(mq[:, k2, :], sps_t[4 * qt + k2])
            mskd_q.append(mq)
        for qt in range(2):
            for k2 in range(4):
                nc.gpsimd.tensor_mul(
                    mskd_q[qt][:, k2, :],
                    mskd_q[qt][:, k2, :],
                    sig_tiles[gi][qt][:, k2, :],
                )
        for qt in range(2, 4):
            mq = mskdp.tile([128, 4, Q], bf16, name="mskd", tag="mskd")
            for k2 in range(4):
                kc = 4 * qt + k2
                nc.vector.tensor_mul(
                    mq[:, k2, :], sps_t[kc], sig_tiles[gi][qt][:, k2, :]
                )
            mskd_q.append(mq)
        del sig_tiles[gi]
        # -- deferred den + normalization for g-1 (after DVE muls) ---------
        if deferred is not None:
            deferred_norm(deferred, deferred_den(deferred))

        # -- ACT: exp eighths into strided attn (finer av unblocking) ------
        attn_t = attnp.tile([128, Q, KC], bf16, name="attn", tag="attn")
        for qt in range(4):
            nc.scalar.activation(
                attn_t[:, :, 4 * qt : 4 * qt + 4],
                mskd_q[qt].rearrange("p c q -> p q c"),
                Act.Exp,
                scale=1.0 / 255.0,
            )

        # -- PE: av (strided rhs) ------------------------------------------
        for kc in range(KC):
            _lab(nc.tensor.matmul(
                ops_t, lhsT=vh[h][:, kc, :], rhs=attn_t[:, :, kc],
                start=(kc == 0), stop=(kc == KC - 1),
            ), "av")
        if h == 3:
            # head-3 pass: finish this group's den+norm immediately so its
            # out-projection stripe unblocks one group later
            finish_deferred((ops_t, h, qs, attn_t))
            deferred = None
        else:
            deferred = (ops_t, h, qs, attn_t)

    # ---------------- tail -------------------------------------------------
    assert deferred is None
    push_outproj(3)
    pop_filler(100.0)

    # release in reverse-allocation order (xtp/wqp already released)
    for p in (woutp, opsp, spsp, projps, ostp, recipp, denp, mskdp, attnp,
              sigp, vp, qkp, persist, const):
        p.release()


def _prep_in_maps(x, W_qkv, b_qkv, W_out, b_out, causal_mask):
    from concurrent.futures import ThreadPoolExecutor

    import ml_dtypes

    bf = ml_dtypes.bfloat16

    def _xT(b):
        return np.ascontiguousarray(x[b].T).astype(bf)

    def _sigT(h):
        # host-side sigmoid (input-only), f32 math, transposed, quantized to
        # u8 (sigma in (0,1): absolute step 1/255; the 1/255 scale is folded
        # into the device exp's scale operand)
        m = causal_mask[h]
        s = 1.0 / (1.0 + np.exp(-m, dtype=np.float32))
        return np.ascontiguousarray(np.rint(s.T * 255.0)).astype(np.uint8)

    def _wq(g):
        # head-major wqkv block [HPC, D, 384] for head group g
        h0 = g * HPC
        out = np.empty((HPC, D, W3), dtype=bf)
        for j in range(HPC):
            h = h0 + j
            cs = slice(h * HD, (h + 1) * HD)
            out[j, :, 0:HD] = (W_qkv[:, cs] * ALPHA).astype(bf)
            out[j, :, HD : 2 * HD] = W_qkv[:, D + h * HD : D + (h + 1) * HD].astype(bf)
            out[j, :, 2 * HD : 3 * HD] = W_qkv[
                :, 2 * D + h * HD : 2 * D + (h + 1) * HD
            ].astype(bf)
        return out

    with ThreadPoolExecutor(16) as ex:
        xT_f = [ex.submit(_xT, b) for b in range(B)]
        sig_f = [ex.submit(_sigT, h) for h in range(H)]
        wq_f = [ex.submit(_wq, g) for g in range(4)]
        xT = [f.result() for f in xT_f]
        sigT = [f.result() for f in sig_f]
        wqh = [f.result() for f in wq_f]

    sig_stack = [
        np.stack([sigT[g * HPC + j] for j in range(HPC)]) for g in range(4)
    ]
    in_maps = []
    for c in range(NCORES):
        b = c // 4
        g = c % 4
        h0 = g * HPC
        bq = np.empty((1, HPC, W3), dtype=bf)
        for j in range(HPC):
            h = h0 + j
            bq[0, j, 0:HD] = (b_qkv[h * HD : (h + 1) * HD] * ALPHA).astype(bf)
            bq[0, j, HD : 2 * HD] = b_qkv[D + h * HD : D + (h + 1) * HD].astype(bf)
            bq[0, j, 2 * HD :] = b_qkv[2 * D + h * HD : 2 * D + (h + 1) * HD].astype(bf)
        in_maps.append(
            {
                "xT": xT[b],
                "wqkvh": wqh[g],
                "bqkvh": bq.reshape(1, HPC * W3),
                "sigT": sig_stack[g],
                "wout": W_out[h0 * HD : (h0 + HPC) * HD, :].astype(bf),
                "bout": (b_out * 0.25).reshape(1, -1).astype(bf),
            }
        )
    return in_maps


def _zero_bias(b_qkv, b_out):
    return bool(not b_qkv.any() and not b_out.any())


def kernel(**inputs):
    x = np.asarray(inputs["x"], dtype=np.float32)
    W_qkv = np.asarray(inputs["W_qkv"], dtype=np.float32)
    b_qkv = np.asarray(inputs["b_qkv"], dtype=np.float32)
    W_out = np.asarray(inputs["W_out"], dtype=np.float32)
    b_out = np.asarray(inputs["b_out"], dtype=np.float32)
    causal_mask = np.asarray(inputs["causal_mask"], dtype=np.float32)

    from concourse.bass_utils import run_bass_kernel_spmd

    nc = _build_program(_zero_bias(b_qkv, b_out))
    in_maps = _prep_in_maps(x, W_qkv, b_qkv, W_out, b_out, causal_mask)
    res = run_bass_kernel_spmd(nc, in_maps, core_ids=list(range(NCORES)))

    out = np.zeros((B, N, D), dtype=np.float32)
    for c in range(NCORES):
        out[c // 4] += np.asarray(res.results[c]["out"]).astype(np.float32)
    return out
